# revision 18
# baseline (speedup 1.0000x reference)
"""Trainium2 Bass kernel for nn_BertLexer (weighted layer mix + ragged segment-mean).

Computation (reference):
    w   = softmax(layer_weights)                       # (L,)
    sub = gamma * einsum('l,lbsf->bsf', w, hidden)     # (B,S,F)
    out[b,w,:] = mean over {s : word_ids[b,s]==w} of sub[b,s,:]   (w >= 1)
    out[b,0,:] = mean over all s of sub[b,s,:]

Strategy (8 NeuronCores, data-parallel over B):
  - Each core gets B/8 = 4 sentences.
  - Layer mix on DVE with 3 scalar_tensor_tensor ops per 128x768 chunk
    via ratio folding:  t1 = h0*(w0/w3) + h3 ; t2 = h1*(w1/w2) + h2 ;
    sub' = t2*(w2/w3) + t1  and the segment matrix absorbs w3*gamma.
  - Segment matrix M[s, w] = w3*gamma/count_w for s in word w's span
    (M[s,0] = w3*gamma/S for all s) is built ON DEVICE by the (otherwise
    idle) GpSimd engine from word ids:  M = (iota_w == ids[p]) * recip[p],
    plus a memset for the dense sentence-mean column 0.
  - Segment mean as an f32r matmul contracting over s on the TensorEngine,
    accumulated in PSUM over the 4 s-chunks of 128, interleaved with the
    mix chunk by chunk.  PSUM -> SBUF copy on ACT/DVE, DMA out.
"""

import numpy as np

L, B, S, F = 4, 32, 512, 768
W_MAX = 256
NW = W_MAX + 1  # 257
NCORES = 8
NB = B // NCORES  # sentences per core
P = 128
SC = S // P  # s-chunks per sentence

_module_cache: dict = {}


def _build_module(r0: float, r1: float, r2: float, col0: float):
    import concourse.bacc as bacc
    import concourse.bass as bass
    import concourse.mybir as mybir
    import concourse.tile as tile

    f32 = mybir.dt.float32
    f32r = mybir.dt.float32r
    i32 = mybir.dt.int32
    mult = mybir.AluOpType.mult
    add = mybir.AluOpType.add
    is_eq = mybir.AluOpType.is_equal

    nc = bacc.Bacc(
        "TRN2", target_bir_lowering=False, debug=False, num_devices=NCORES
    )
    hid = nc.dram_tensor("hid", (L, NB, S, F), f32, kind="ExternalInput").ap()
    # aux[b, p, c, 0] = word id at position s=c*P+p (as float)
    # aux[b, p, c, 1] = scale/count for that position (0 for pad ids)
    aux = nc.dram_tensor("aux", (NB, P, SC, 2), f32, kind="ExternalInput").ap()
    out = nc.dram_tensor("out", (NB, NW, F), f32, kind="ExternalOutput").ap()

    wtiles = [(0, 128), (128, 256), (256, 257)]
    fsplits = [(0, 384), (384, 768)]

    with tile.TileContext(nc) as tc:
        with (
            tc.tile_pool(name="const", bufs=1) as cpool,
            tc.tile_pool(name="h", bufs=20) as hpool,
            tc.tile_pool(name="t", bufs=4) as tpool,
            tc.tile_pool(name="sub", bufs=8) as spool,
            tc.tile_pool(name="m", bufs=8) as mpool,
            tc.tile_pool(name="o", bufs=8) as opool,
            tc.tile_pool(name="ps", bufs=8, space=bass.MemorySpace.PSUM) as pspool,
        ):
            iota_t = cpool.tile([P, NW], i32, tag="iota")
            nc.gpsimd.iota(iota_t[:], pattern=[[1, NW]], channel_multiplier=0)
            auxs = []
            for b in range(NB):
                at = cpool.tile([P, SC, 2], f32, tag=f"aux{b}", name=f"aux{b}")
                nc.sync.dma_start(at[:], aux[b])
                auxs.append(at)
            for b in range(NB):
                # build this sentence's segment matrix chunks on GpSimd
                mcs = []
                for c in range(SC):
                    mc = mpool.tile([P, NW], f32r, tag="m", name=f"mc{b}_{c}")
                    nc.gpsimd.tensor_scalar(
                        mc[:],
                        iota_t[:],
                        auxs[b][:, c, 0:1],
                        auxs[b][:, c, 1:2],
                        op0=is_eq,
                        op1=mult,
                    )
                    nc.gpsimd.tensor_scalar(
                        mc[:, 0:1],
                        iota_t[:, 0:1],
                        0.0,
                        col0,
                        op0=mult,
                        op1=add,
                    )
                    mcs.append(mc)
                ps_tiles = {}
                for t in range(len(wtiles)):
                    for fi in range(len(fsplits)):
                        ps_tiles[t, fi] = pspool.tile(
                            [P, 384], f32, tag="ps", name=f"ps{b}_{t}_{fi}"
                        )
                for c in range(SC):
                    hts = []
                    for l in range(L):
                        ht = hpool.tile([P, F], f32, tag="h", name=f"h{b}_{c}_{l}")
                        # alternate between the two HWDGE rings (SP / ACT)
                        eng = nc.sync if (c * L + l) % 2 == 0 else nc.scalar
                        eng.dma_start(ht[:], hid[l, b, c * P : (c + 1) * P, :])
                        hts.append(ht)
                    t1 = tpool.tile([P, F], f32, tag="t")
                    nc.vector.scalar_tensor_tensor(
                        t1[:], hts[0][:], float(r0), hts[3][:], op0=mult, op1=add
                    )
                    t2 = tpool.tile([P, F], f32, tag="t")
                    nc.vector.scalar_tensor_tensor(
                        t2[:], hts[1][:], float(r1), hts[2][:], op0=mult, op1=add
                    )
                    sub = spool.tile([P, F], f32r, tag="sub")
                    nc.vector.scalar_tensor_tensor(
                        sub[:], t2[:], float(r2), t1[:], op0=mult, op1=add
                    )
                    # accumulate this chunk into all word-tiles right away
                    for t, (w0, w1) in enumerate(wtiles):
                        msz = w1 - w0
                        for fi, (f0, f1) in enumerate(fsplits):
                            nc.tensor.matmul(
                                ps_tiles[t, fi][0:msz, 0 : f1 - f0],
                                mcs[c][:, w0:w1],
                                sub[:, f0:f1],
                                start=(c == 0),
                                stop=(c == SC - 1),
                            )
                for t, (w0, w1) in enumerate(wtiles):
                    msz = w1 - w0
                    for fi, (f0, f1) in enumerate(fsplits):
                        ob = opool.tile([P, 384], f32, tag="o")
                        nc.any.tensor_copy(ob[0:msz, :], ps_tiles[t, fi][0:msz, :])
                        nc.scalar.dma_start(out[b, w0:w1, f0:f1], ob[0:msz, :])

    nc.compile()
    return nc


def _prepare(hidden_states, layer_weights, gamma, word_ids):
    """Host-side prep: softmax ratios + per-position recip table + shards."""
    hidden_states = np.ascontiguousarray(hidden_states, dtype=np.float32)
    lw = np.asarray(layer_weights, dtype=np.float64)
    g = float(np.asarray(gamma, dtype=np.float64).reshape(-1)[0])
    ids = np.asarray(word_ids)

    e = np.exp(lw - lw.max())
    w = e / e.sum()  # softmax, float64
    r0 = float(w[0] / w[3])
    r1 = float(w[1] / w[2])
    r2 = float(w[2] / w[3])
    scale = float(w[3] * g)  # absorbed into M
    col0 = float(np.float32(scale / S))

    # aux[b, s, 0] = ids as float; aux[b, s, 1] = scale/count (0 for pad)
    auxm = np.zeros((B, S, 2), dtype=np.float64)
    for b in range(B):
        counts = np.bincount(ids[b], minlength=NW).astype(np.float64)
        recip = np.zeros(NW, dtype=np.float64)
        nz = counts > 0
        recip[nz] = scale / counts[nz]
        recip[0] = 0.0  # pad positions contribute only via column 0
        auxm[b, :, 0] = ids[b]
        auxm[b, :, 1] = recip[ids[b]]
    # reorder to [b, p, c, 2] with s = c*P + p
    auxm = auxm.reshape(B, SC, P, 2).transpose(0, 2, 1, 3)
    auxm = np.ascontiguousarray(auxm, dtype=np.float32)

    in_maps = []
    for i in range(NCORES):
        bs = slice(i * NB, (i + 1) * NB)
        in_maps.append(
            {
                "hid": np.ascontiguousarray(hidden_states[:, bs]),
                "aux": np.ascontiguousarray(auxm[bs]),
            }
        )
    return (r0, r1, r2, col0), in_maps


def _run(inputs: dict, trace: bool = False):
    from concourse.bass_utils import run_bass_kernel_spmd

    params, in_maps = _prepare(**inputs)
    if params not in _module_cache:
        _module_cache[params] = _build_module(*params)
    nc = _module_cache[params]

    res = run_bass_kernel_spmd(
        nc, in_maps, core_ids=list(range(NCORES)), trace=trace
    )
    out = np.concatenate([r["out"] for r in res.results], axis=0)
    return out, res


def kernel(**inputs) -> np.ndarray:
    out, _ = _run(inputs, trace=False)
    return out


# revision 20
# speedup vs baseline: 1.1646x; 1.1646x over previous
"""Trainium2 Bass kernel for nn_BertLexer (weighted layer mix + ragged segment-mean).

Computation (reference):
    w   = softmax(layer_weights)                       # (L,)
    sub = gamma * einsum('l,lbsf->bsf', w, hidden)     # (B,S,F)
    out[b,w,:] = mean over {s : word_ids[b,s]==w} of sub[b,s,:]   (w >= 1)
    out[b,0,:] = mean over all s of sub[b,s,:]

Strategy (8 NeuronCores, data-parallel over B):
  - Each core gets B/8 = 4 sentences.
  - Layer mix on DVE with 3 scalar_tensor_tensor ops per 128x768 chunk
    via ratio folding:  t1 = h0*(w0/w3) + h3 ; t2 = h1*(w1/w2) + h2 ;
    sub' = t2*(w2/w3) + t1  and the segment matrix absorbs w3*gamma.
  - Segment matrix M[s, w] = w3*gamma/count_w for s in word w's span
    (M[s,0] = w3*gamma/S for all s) is built ON DEVICE by the (otherwise
    idle) GpSimd engine from word ids:  M = (iota_w == ids[p]) * recip[p],
    plus a memset for the dense sentence-mean column 0.
  - Segment mean as an f32r matmul contracting over s on the TensorEngine,
    accumulated in PSUM over the 4 s-chunks of 128, interleaved with the
    mix chunk by chunk.  PSUM -> SBUF copy on ACT/DVE, DMA out.
"""

import numpy as np

L, B, S, F = 4, 32, 512, 768
W_MAX = 256
NW = W_MAX + 1  # 257
NCORES = 8
NB = B // NCORES  # sentences per core
P = 128
SC = S // P  # s-chunks per sentence

_module_cache: dict = {}


def _build_module(r0: float, r1: float, r2: float, col0: float, order):
    import concourse.bacc as bacc
    import concourse.bass as bass
    import concourse.mybir as mybir
    import concourse.tile as tile

    f32 = mybir.dt.float32
    f32r = mybir.dt.float32r
    i32 = mybir.dt.int32
    mult = mybir.AluOpType.mult
    add = mybir.AluOpType.add
    is_eq = mybir.AluOpType.is_equal

    nc = bacc.Bacc(
        "TRN2", target_bir_lowering=False, debug=False, num_devices=NCORES
    )
    hid = nc.dram_tensor("hid", (L, NB, S, F), f32, kind="ExternalInput").ap()
    mm = nc.dram_tensor("mm", (NB, P, SC, NW), f32r, kind="ExternalInput").ap()
    out = nc.dram_tensor("out", (NB, NW, F), f32, kind="ExternalOutput").ap()

    wtiles = [(0, 128), (128, 256), (256, 257)]
    fsplits = [(0, 384), (384, 768)]

    with tile.TileContext(nc) as tc:
        with (
            tc.tile_pool(name="const", bufs=1) as cpool,
            tc.tile_pool(name="h", bufs=20) as hpool,
            tc.tile_pool(name="t", bufs=4) as tpool,
            tc.tile_pool(name="sub", bufs=8) as spool,
            tc.tile_pool(name="m", bufs=4) as mpool,
            tc.tile_pool(name="o", bufs=8) as opool,
            tc.tile_pool(name="ps", bufs=8, space=bass.MemorySpace.PSUM) as pspool,
        ):
            mts = []
            for b in range(NB):
                mt = mpool.tile([P, SC, NW], f32r, tag="m", name=f"mt{b}")
                nc.sync.dma_start(mt[:], mm[b])
                mts.append(mt)
            for b in range(NB):
                mcs = [mts[b][:, c, :] for c in range(SC)]
                ps_tiles = {}
                for t in range(len(wtiles)):
                    for fi in range(len(fsplits)):
                        ps_tiles[t, fi] = pspool.tile(
                            [P, 384], f32, tag="ps", name=f"ps{b}_{t}_{fi}"
                        )
                for c in range(SC):
                    hts = []
                    for l in range(L):
                        ht = hpool.tile([P, F], f32, tag="h", name=f"h{b}_{c}_{l}")
                        # alternate between the two HWDGE rings (SP / ACT)
                        eng = nc.sync if (c * L + l) % 2 == 0 else nc.scalar
                        eng.dma_start(ht[:], hid[l, b, c * P : (c + 1) * P, :])
                        hts.append(ht)
                    ia, ib, ic, id_ = order
                    t1 = tpool.tile([P, F], f32, tag="t")
                    nc.vector.scalar_tensor_tensor(
                        t1[:], hts[ia][:], float(r0), hts[id_][:],
                        op0=mult, op1=add,
                    )
                    t2 = tpool.tile([P, F], f32, tag="t")
                    nc.vector.scalar_tensor_tensor(
                        t2[:], hts[ib][:], float(r1), hts[ic][:],
                        op0=mult, op1=add,
                    )
                    sub = spool.tile([P, F], f32r, tag="sub")
                    nc.vector.scalar_tensor_tensor(
                        sub[:], t2[:], float(r2), t1[:], op0=mult, op1=add
                    )
                    # accumulate this chunk into all word-tiles right away
                    for t, (w0, w1) in enumerate(wtiles):
                        msz = w1 - w0
                        for fi, (f0, f1) in enumerate(fsplits):
                            nc.tensor.matmul(
                                ps_tiles[t, fi][0:msz, 0 : f1 - f0],
                                mcs[c][:, w0:w1],
                                sub[:, f0:f1],
                                start=(c == 0),
                                stop=(c == SC - 1),
                            )
                for t, (w0, w1) in enumerate(wtiles):
                    msz = w1 - w0
                    for fi, (f0, f1) in enumerate(fsplits):
                        ob = opool.tile([P, 384], f32, tag="o")
                        nc.any.tensor_copy(ob[0:msz, :], ps_tiles[t, fi][0:msz, :])
                        nc.scalar.dma_start(out[b, w0:w1, f0:f1], ob[0:msz, :])

    nc.compile()
    return nc


def _prepare(hidden_states, layer_weights, gamma, word_ids):
    """Host-side prep: softmax ratios + per-position recip table + shards."""
    hidden_states = np.ascontiguousarray(hidden_states, dtype=np.float32)
    lw = np.asarray(layer_weights, dtype=np.float64)
    g = float(np.asarray(gamma, dtype=np.float64).reshape(-1)[0])
    ids = np.asarray(word_ids)

    e = np.exp(lw - lw.max())
    w = e / e.sum()  # softmax, float64
    # pair layers sorted by weight so every folded ratio is <= 1:
    #   sub*w[d] = w[a]h[a] + w[b]h[b] + w[c]h[c] + w[d]h[d]
    order = tuple(int(i) for i in np.argsort(w))
    ia, ib, ic, id_ = order
    r0 = float(w[ia] / w[id_])
    r1 = float(w[ib] / w[ic]) if w[ic] > 0 else 0.0
    r2 = float(w[ic] / w[id_])
    scale = float(w[id_] * g)  # absorbed into M
    col0 = float(np.float32(scale / S))

    mmat = np.zeros((B, S, NW), dtype=np.float64)
    rows = np.arange(S)
    for b in range(B):
        counts = np.bincount(ids[b], minlength=NW).astype(np.float64)
        recip = np.zeros(NW, dtype=np.float64)
        nz = counts > 0
        recip[nz] = scale / counts[nz]
        sel = ids[b] > 0
        mmat[b, rows[sel], ids[b][sel]] = recip[ids[b][sel]]
        mmat[b, :, 0] = scale / S
    mmat = mmat.reshape(B, SC, P, NW).transpose(0, 2, 1, 3)
    mmat = np.ascontiguousarray(mmat, dtype=np.float32)

    in_maps = []
    for i in range(NCORES):
        bs = slice(i * NB, (i + 1) * NB)
        in_maps.append(
            {
                "hid": np.ascontiguousarray(hidden_states[:, bs]),
                "mm": np.ascontiguousarray(mmat[bs]),
            }
        )
    return (r0, r1, r2, col0, order), in_maps


def _run(inputs: dict, trace: bool = False):
    from concourse.bass_utils import run_bass_kernel_spmd

    params, in_maps = _prepare(**inputs)
    if params not in _module_cache:
        _module_cache[params] = _build_module(*params)
    nc = _module_cache[params]

    res = run_bass_kernel_spmd(
        nc, in_maps, core_ids=list(range(NCORES)), trace=trace
    )
    out = np.concatenate([r["out"] for r in res.results], axis=0)
    return out, res


def kernel(**inputs) -> np.ndarray:
    out, _ = _run(inputs, trace=False)
    return out
